# revision 5
# baseline (speedup 1.0000x reference)
"""Trainium2 Bass kernel for nn_BuildCost (light-field cost volume).

out[b, co, d, i, j] = (1/mask_avg[i,j]) * sum_{p,q} W[co, p*9+q]
                       * mask[p*9+q, i, j] * x[b, co//4, p*9+q, i+d*(4-p), j+d*(4-q)]

Sharding: 8 cores, each owns a 24-row band of the 192-row output.
Per core, d-outer loop; for each (d, half-band) the 81 angular views are
processed as 21 K-chunks (4 views x 32 channels = 128 partitions) of a
block-diagonal grouped-conv matmul accumulated in PSUM.  The per-view
spatial shift is applied by the HBM->SBUF DMA (reads a shifted window of
the host-padded per-core x slice), mask modulation (pre-normalized by
mask_avg on host) is a single [128, N] bf16 DVE multiply per chunk.
"""

import sys

sys.path.insert(0, "/opt/trn_rl_repo")

import numpy as np
import ml_dtypes

A = 9
C0 = 4          # A // 2
BDR = 16        # C0 * MAXD
H = W_IMG = 192
CIN = 32
COUT = 128
M_PER_G = 4     # COUT // CIN
ND = 9          # disparities -4..4
N_CORES = 8
BAND = H // N_CORES          # 24 output rows per core
HALF = BAND // 2             # 12 rows per half-band
NPIX = HALF * W_IMG          # 2304 pixels per half-band
NCHUNK = 21                  # ceil(81 / 4) view-chunks
XROWS = BAND + 2 * BDR       # 56 rows of padded x per core
XCOLS = W_IMG + 2 * BDR      # 224 padded cols

_BF16 = ml_dtypes.bfloat16
_PROGRAM = None


def _build_program():
    import concourse.bacc as bacc
    import concourse.tile as tile
    from concourse import mybir

    nc = bacc.Bacc("TRN2", target_bir_lowering=False, debug=False,
                   num_devices=N_CORES)

    xd = nc.dram_tensor("x_core", [81 * CIN, XROWS, XCOLS], mybir.dt.bfloat16,
                        kind="ExternalInput").ap()
    md = nc.dram_tensor("mask_core", [81 * CIN, BAND, W_IMG], mybir.dt.bfloat16,
                        kind="ExternalInput").ap()
    wd = nc.dram_tensor("wt", [NCHUNK, 128, 128], mybir.dt.bfloat16,
                        kind="ExternalInput").ap()
    od = nc.dram_tensor("out", [COUT, ND, BAND, W_IMG], mybir.dt.float32,
                        kind="ExternalOutput").ap()

    with tile.TileContext(nc) as tc:
        with (
            tc.tile_pool(name="wpool", bufs=1) as wpool,
            tc.tile_pool(name="mpool", bufs=1) as mpool,
            tc.tile_pool(name="xspool", bufs=4) as xspool,
            tc.tile_pool(name="xmpool", bufs=3) as xmpool,
            tc.tile_pool(name="opool", bufs=2) as opool,
            tc.tile_pool(name="psum", bufs=1, space="PSUM") as psumpool,
        ):
            w_tiles = []
            for c in range(NCHUNK):
                wt = wpool.tile([128, 128], mybir.dt.bfloat16, tag=f"w{c}")
                nc.sync.dma_start(out=wt[:], in_=wd[c])
                w_tiles.append(wt)

            for half in range(2):
                r0 = HALF * half
                # mask chunk tiles for this half-band, resident across d loop
                m_tiles = []
                for c in range(NCHUNK):
                    rows = 128 if c < NCHUNK - 1 else 32
                    mt = mpool.tile([rows, HALF, W_IMG], mybir.dt.bfloat16,
                                    tag=f"m{c}")
                    nc.sync.dma_start(
                        out=mt[:],
                        in_=md[c * 128:c * 128 + rows, r0:r0 + HALF, :])
                    m_tiles.append(mt)

                for d in range(ND):
                    dd = d - 4
                    ps = psumpool.tile([128, NPIX], mybir.dt.float32)
                    for c in range(NCHUNK):
                        npq = 4 if c < NCHUNK - 1 else 1
                        K = 32 * npq
                        # full 224-wide rows, written at col offset 32-cs so
                        # the needed 192 cols sit at a fixed aligned offset 32
                        xs = xspool.tile([128, HALF, 256], mybir.dt.bfloat16,
                                         tag="xs")
                        for pql in range(npq):
                            pq = 4 * c + pql
                            p, q = divmod(pq, A)
                            rs = r0 + BDR + dd * (4 - p)
                            cs = BDR + dd * (4 - q)
                            eng = nc.sync if pq % 2 == 0 else nc.scalar
                            eng.dma_start(
                                out=xs[32 * pql:32 * pql + 32, :,
                                       32 - cs:32 - cs + XCOLS],
                                in_=xd[pq * 32:(pq + 1) * 32,
                                       rs:rs + HALF, :])
                        xm = xmpool.tile([128, HALF, W_IMG], mybir.dt.bfloat16,
                                         tag="xm")
                        nc.vector.tensor_mul(
                            xm[:K, :, :],
                            xs[:K, :, 32:32 + W_IMG],
                            m_tiles[c][:K, :, :])
                        xm2 = xm[:].rearrange("p a b -> p (a b)")
                        for n0 in range(0, NPIX, 512):
                            n1 = min(NPIX, n0 + 512)
                            nc.tensor.matmul(
                                ps[:, n0:n1],
                                w_tiles[c][:K, :],
                                xm2[:K, n0:n1],
                                start=(c == 0),
                                stop=(c == NCHUNK - 1),
                            )
                    osb = opool.tile([128, NPIX], mybir.dt.float32, tag="osb")
                    nc.any.tensor_copy(osb[:], ps[:])
                    nc.sync.dma_start(
                        out=od[:, d, r0:r0 + HALF, :],
                        in_=osb[:].rearrange("p (a b) -> p a b", a=HALF))

    nc.compile()
    return nc


def _get_program():
    global _PROGRAM
    if _PROGRAM is None:
        _PROGRAM = _build_program()
    return _PROGRAM


def _host_prep(x, mask, W):
    # x: [1, 32, 81, 192, 192] f32 -> padded pq-major bf16 [81, 32, 224, 224]
    xt = np.ascontiguousarray(x[0].transpose(1, 0, 2, 3))  # [81, 32, 192, 192]
    xp = np.zeros((81, CIN, XCOLS, XCOLS), dtype=_BF16)
    xp[:, :, BDR:BDR + H, BDR:BDR + W_IMG] = xt

    m = mask[0].astype(np.float32)                     # [81, 192, 192]
    mask_n = (m / m.mean(axis=0, keepdims=True)).astype(_BF16)

    # block-diagonal grouped-conv weights, pq-major chunks of 4 views
    wt = np.zeros((NCHUNK, 128, 128), dtype=np.float32)
    co = np.arange(COUT)
    for c in range(NCHUNK):
        npq = 4 if c < NCHUNK - 1 else 1
        for pql in range(npq):
            pq = 4 * c + pql
            g = co // M_PER_G
            wt[c, pql * 32 + g, co] = W[co, pq]
    wt = wt.astype(_BF16)

    in_maps = []
    for k in range(N_CORES):
        x_core = np.ascontiguousarray(
            xp[:, :, BAND * k:BAND * k + XROWS, :]).reshape(
                81 * CIN, XROWS, XCOLS)
        m_band = mask_n[:, BAND * k:BAND * k + BAND, :]           # [81,24,192]
        m_core = np.ascontiguousarray(
            np.broadcast_to(m_band[:, None, :, :],
                            (81, CIN, BAND, W_IMG))).reshape(
                                81 * CIN, BAND, W_IMG)
        in_maps.append({"x_core": x_core, "mask_core": m_core, "wt": wt})
    return in_maps


def kernel(x, mask, W):
    from concourse.bass_utils import run_bass_kernel_spmd

    nc = _get_program()
    in_maps = _host_prep(np.asarray(x), np.asarray(mask), np.asarray(W))
    res = run_bass_kernel_spmd(nc, in_maps, list(range(N_CORES)))

    out = np.empty((1, COUT, ND, H, W_IMG), dtype=np.float32)
    for k in range(N_CORES):
        out[0, :, :, BAND * k:BAND * k + BAND, :] = res.results[k]["out"]
    return out


# revision 6
# speedup vs baseline: 1.3005x; 1.3005x over previous
"""Trainium2 Bass kernel for nn_BuildCost (light-field cost volume).

out[b, co, d, i, j] = (1/mask_avg[i,j]) * sum_{p,q} W[co, p*9+q]
                       * mask[p*9+q, i, j] * x[b, co//4, p*9+q, i+d*(4-p), j+d*(4-q)]

Sharding: 8 cores, each owns a 24-row band of the 192-row output.
Per core, d-outer loop; for each (d, half-band) the 81 angular views are
processed as 21 K-chunks (4 views x 32 channels = 128 partitions) of a
block-diagonal grouped-conv matmul accumulated in PSUM.  The per-view
spatial shift is applied by the HBM->SBUF DMA (reads a shifted window of
the host-padded per-core x slice), mask modulation (pre-normalized by
mask_avg on host) is a single [128, N] bf16 DVE multiply per chunk.
"""

import sys

sys.path.insert(0, "/opt/trn_rl_repo")

import numpy as np
import ml_dtypes

A = 9
C0 = 4          # A // 2
BDR = 16        # C0 * MAXD
H = W_IMG = 192
CIN = 32
COUT = 128
M_PER_G = 4     # COUT // CIN
ND = 9          # disparities -4..4
N_CORES = 8
BAND = H // N_CORES          # 24 output rows per core
HALF = BAND // 2             # 12 rows per half-band
NPIX = HALF * W_IMG          # 2304 pixels per half-band
NCHUNK = 21                  # ceil(81 / 4) view-chunks
XROWS = BAND + 2 * BDR       # 56 rows of padded x per core
XCOLS = W_IMG + 2 * BDR      # 224 padded cols

_BF16 = ml_dtypes.bfloat16
_PROGRAM = None


def _build_program():
    import concourse.bacc as bacc
    import concourse.tile as tile
    from concourse import mybir

    nc = bacc.Bacc("TRN2", target_bir_lowering=False, debug=False,
                   num_devices=N_CORES)

    xd = nc.dram_tensor("x_core", [81 * CIN, XROWS, XCOLS], mybir.dt.bfloat16,
                        kind="ExternalInput").ap()
    md = nc.dram_tensor("mask_core", [81 * CIN, BAND, W_IMG], mybir.dt.bfloat16,
                        kind="ExternalInput").ap()
    wd = nc.dram_tensor("wt", [NCHUNK, 128, 128], mybir.dt.bfloat16,
                        kind="ExternalInput").ap()
    od = nc.dram_tensor("out", [COUT, ND, BAND, W_IMG], mybir.dt.float32,
                        kind="ExternalOutput").ap()

    with tile.TileContext(nc) as tc:
        with (
            tc.tile_pool(name="wpool", bufs=1) as wpool,
            tc.tile_pool(name="mpool", bufs=1) as mpool,
            tc.tile_pool(name="xspool", bufs=4) as xspool,
            tc.tile_pool(name="xmpool", bufs=3) as xmpool,
            tc.tile_pool(name="opool", bufs=2) as opool,
            tc.tile_pool(name="psum", bufs=1, space="PSUM") as psumpool,
        ):
            w_tiles = []
            for c in range(NCHUNK):
                wt = wpool.tile([128, 128], mybir.dt.bfloat16, tag=f"w{c}")
                nc.sync.dma_start(out=wt[:], in_=wd[c])
                w_tiles.append(wt)

            for half in range(2):
                r0 = HALF * half
                # mask chunk tiles for this half-band, resident across d loop
                m_tiles = []
                for c in range(NCHUNK):
                    rows = 128 if c < NCHUNK - 1 else 32
                    mt = mpool.tile([rows, HALF, W_IMG], mybir.dt.bfloat16,
                                    tag=f"m{c}")
                    nc.sync.dma_start(
                        out=mt[:],
                        in_=md[c * 128:c * 128 + rows, r0:r0 + HALF, :])
                    m_tiles.append(mt)

                for d in range(ND):
                    dd = d - 4
                    ps = psumpool.tile([128, NPIX], mybir.dt.float32)
                    for c in range(NCHUNK):
                        npq = 4 if c < NCHUNK - 1 else 1
                        K = 32 * npq
                        # full 224-wide row-blocks written contiguously at a
                        # per-view flat base (32-cs), so both DMA sides are
                        # one 5.4KB run and the needed 192 cols of every view
                        # sit at the same aligned offset 32 with stride 224
                        xs = xspool.tile([128, 32 + HALF * XCOLS],
                                         mybir.dt.bfloat16, tag="xs")
                        for pql in range(npq):
                            pq = 4 * c + pql
                            p, q = divmod(pq, A)
                            rs = r0 + BDR + dd * (4 - p)
                            cs = BDR + dd * (4 - q)
                            eng = nc.sync if pq % 2 == 0 else nc.scalar
                            eng.dma_start(
                                out=xs[32 * pql:32 * pql + 32,
                                       32 - cs:32 - cs + HALF * XCOLS],
                                in_=xd[pq * 32:(pq + 1) * 32,
                                       rs:rs + HALF, :])
                        xm = xmpool.tile([128, HALF, W_IMG], mybir.dt.bfloat16,
                                         tag="xm")
                        xsv = xs[:, 32:].rearrange(
                            "p (a b) -> p a b", a=HALF)[:, :, :W_IMG]
                        nc.vector.tensor_mul(
                            xm[:K, :, :], xsv[:K, :, :], m_tiles[c][:K, :, :])
                        xm2 = xm[:].rearrange("p a b -> p (a b)")
                        for n0 in range(0, NPIX, 512):
                            n1 = min(NPIX, n0 + 512)
                            nc.tensor.matmul(
                                ps[:, n0:n1],
                                w_tiles[c][:K, :],
                                xm2[:K, n0:n1],
                                start=(c == 0),
                                stop=(c == NCHUNK - 1),
                            )
                    osb = opool.tile([128, NPIX], mybir.dt.float32, tag="osb")
                    nc.any.tensor_copy(osb[:], ps[:])
                    nc.sync.dma_start(
                        out=od[:, d, r0:r0 + HALF, :],
                        in_=osb[:].rearrange("p (a b) -> p a b", a=HALF))

    nc.compile()
    return nc


def _get_program():
    global _PROGRAM
    if _PROGRAM is None:
        _PROGRAM = _build_program()
    return _PROGRAM


def _host_prep(x, mask, W):
    # x: [1, 32, 81, 192, 192] f32 -> padded pq-major bf16 [81, 32, 224, 224]
    xt = np.ascontiguousarray(x[0].transpose(1, 0, 2, 3))  # [81, 32, 192, 192]
    xp = np.zeros((81, CIN, XCOLS, XCOLS), dtype=_BF16)
    xp[:, :, BDR:BDR + H, BDR:BDR + W_IMG] = xt

    m = mask[0].astype(np.float32)                     # [81, 192, 192]
    mask_n = (m / m.mean(axis=0, keepdims=True)).astype(_BF16)

    # block-diagonal grouped-conv weights, pq-major chunks of 4 views
    wt = np.zeros((NCHUNK, 128, 128), dtype=np.float32)
    co = np.arange(COUT)
    for c in range(NCHUNK):
        npq = 4 if c < NCHUNK - 1 else 1
        for pql in range(npq):
            pq = 4 * c + pql
            g = co // M_PER_G
            wt[c, pql * 32 + g, co] = W[co, pq]
    wt = wt.astype(_BF16)

    in_maps = []
    for k in range(N_CORES):
        x_core = np.ascontiguousarray(
            xp[:, :, BAND * k:BAND * k + XROWS, :]).reshape(
                81 * CIN, XROWS, XCOLS)
        m_band = mask_n[:, BAND * k:BAND * k + BAND, :]           # [81,24,192]
        m_core = np.ascontiguousarray(
            np.broadcast_to(m_band[:, None, :, :],
                            (81, CIN, BAND, W_IMG))).reshape(
                                81 * CIN, BAND, W_IMG)
        in_maps.append({"x_core": x_core, "mask_core": m_core, "wt": wt})
    return in_maps


def kernel(x, mask, W):
    from concourse.bass_utils import run_bass_kernel_spmd

    nc = _get_program()
    in_maps = _host_prep(np.asarray(x), np.asarray(mask), np.asarray(W))
    res = run_bass_kernel_spmd(nc, in_maps, list(range(N_CORES)))

    out = np.empty((1, COUT, ND, H, W_IMG), dtype=np.float32)
    for k in range(N_CORES):
        out[0, :, :, BAND * k:BAND * k + BAND, :] = res.results[k]["out"]
    return out
